# revision 3
# baseline (speedup 1.0000x reference)
"""Bass/Tile kernel for nn_Attention_89103391523461 (sparse talking-heads attention).

Sharding: 8 cores = (batch in {0,1}) x (4 output-head groups of 4 heads).
Talking-heads pre-softmax mix is folded into per-output-head queries
(Q''_k[(h,d)] = pre_proj[h,k]*SCALE*q[(h,d)]), making dots a K=1024 contraction.
Causal structure limits each 128-row query tile t to jlen = 16 + 128*(t+1)
key columns (16 mem slots + seq keys). Top-64 threshold per row via
max8/match_replace (direct for short rows, segmented two-level for long).
Partial out-projection per head group + grouped AllReduce at the end.
"""
import numpy as np
import ml_dtypes

import concourse.bass as bass
import concourse.bacc as bacc
import concourse.mybir as mybir
from concourse.tile import TileContext
from concourse.bass_utils import run_bass_kernel_spmd

F32 = mybir.dt.float32
BF16 = mybir.dt.bfloat16
AF = mybir.ActivationFunctionType
ALU = mybir.AluOpType

B, N, DIM = 2, 2048, 1024
H, DH = 16, 64
M = 16
TOPK = 64
SCALE = DH ** -0.5
G = 4                 # heads per core
NT = N // 128         # 16 row tiles
NEG = -1e30


def jlen_of(t):
    return M + 128 * (t + 1)


def build(nc: bass.Bass, use_cc=True, use_dmat=True, use_dmat_fin=True, batch_dots=False):
    # ---------- DRAM I/O ----------
    xT = nc.dram_tensor("xT", [8, 128, N], BF16, kind="ExternalInput")
    wq = nc.dram_tensor("wq", [8, 128, DIM], BF16, kind="ExternalInput")
    wkvk = nc.dram_tensor("wkvk", [8, 128, DIM], BF16, kind="ExternalInput")
    wkvv = nc.dram_tensor("wkvv", [8, 128, G * DH], BF16, kind="ExternalInput")
    memkT = nc.dram_tensor("memkT", [8, 128, M], BF16, kind="ExternalInput")
    memv = nc.dram_tensor("memv", [M, G * DH], BF16, kind="ExternalInput")
    ppv = nc.dram_tensor("ppv", [8, 128, G], F32, kind="ExternalInput")
    wout = nc.dram_tensor("wout", [2, 128, DIM], BF16, kind="ExternalInput")
    bout = nc.dram_tensor("bout", [128, DIM], BF16, kind="ExternalInput")
    bandm = nc.dram_tensor("bandm", [128, 128], BF16, kind="ExternalInput")
    ident = nc.dram_tensor("ident", [128, 128], BF16, kind="ExternalInput")
    y = nc.dram_tensor("y", [NT, 128, DIM], F32, kind="ExternalOutput")

    ytp = nc.dram_tensor("ytp", [NT, 8, 128, 128], BF16)                       # partial y^T
    ytr = nc.dram_tensor("ytr", [NT, 8, 128, 128], BF16)  # reduced y^T

    with TileContext(nc) as tc:
        with (
            tc.tile_pool(name="const", bufs=1) as cpool,
            tc.tile_pool(name="psD", bufs=3, space="PSUM") as psD,
            tc.tile_pool(name="psS", bufs=2, space="PSUM") as psS,
            tc.tile_pool(name="psO", bufs=3, space="PSUM") as psO,
        ):
            # ---------- load constants / weights ----------
            ppool_cm = tc.tile_pool(name="proj", bufs=1)
            ppool = ppool_cm.__enter__()
            _ = ppool
            xT_sb = ppool.tile([128, 8, N], BF16)
            wq_sb = ppool.tile([128, 8, DIM], BF16)
            wkvk_sb = ppool.tile([128, 8, DIM], BF16)
            wkvv_sb = ppool.tile([128, 8, G * DH], BF16)
            ppv_sb = cpool.tile([128, 8 * G], F32)
            wout_sb = cpool.tile([128, 2, DIM], BF16)
            bout_sb = cpool.tile([128, DIM], BF16)
            bandm_sb = cpool.tile([128, 128], BF16)
            ident_sb = cpool.tile([128, 128], BF16)
            KT_sb = cpool.tile([128, 8, M + N], BF16)     # [(hd)chunk, m, j]
            V_sb = cpool.tile([128, NT + 1, G * DH], BF16)  # chunk 0 = mem rows
            qT_sb = cpool.tile([128, 8, N], BF16)

            for m in range(8):
                nc.sync.dma_start(xT_sb[:, m, :], xT[m])
                nc.sync.dma_start(wq_sb[:, m, :], wq[m])
                nc.sync.dma_start(wkvk_sb[:, m, :], wkvk[m])
                nc.sync.dma_start(wkvv_sb[:, m, :], wkvv[m])
                nc.sync.dma_start(KT_sb[:, m, 0:M], memkT[m])
                nc.sync.dma_start(ppv_sb[:, m * G:(m + 1) * G], ppv[m])
            nc.sync.dma_start(V_sb[0:M, 0, :], memv[:, :])
            for kc in range(2):
                nc.sync.dma_start(wout_sb[:, kc, :], wout[kc])
            nc.sync.dma_start(bout_sb[:, :], bout[:, :])
            nc.sync.dma_start(bandm_sb[:, :], bandm[:, :])
            nc.sync.dma_start(ident_sb[:, :], ident[:, :])

            # ---------- projections ----------
            # jq-outer so early row tiles' K^T/q^T columns land first
            for jq in range(4):
                for m in range(8):
                    ps = psD.tile([128, 512], F32, tag="psd")
                    for dc in range(8):
                        nc.tensor.matmul(
                            ps[:, :], wq_sb[:, dc, m * 128:(m + 1) * 128],
                            xT_sb[:, dc, jq * 512:(jq + 1) * 512],
                            start=(dc == 0), stop=(dc == 7))
                    nc.scalar.activation(qT_sb[:, m, jq * 512:(jq + 1) * 512],
                                         ps[:, :], AF.Copy)
                for m in range(8):
                    ps = psD.tile([128, 512], F32, tag="psd")
                    for dc in range(8):
                        nc.tensor.matmul(
                            ps[:, :], wkvk_sb[:, dc, m * 128:(m + 1) * 128],
                            xT_sb[:, dc, jq * 512:(jq + 1) * 512],
                            start=(dc == 0), stop=(dc == 7))
                    nc.scalar.activation(KT_sb[:, m, M + jq * 512: M + (jq + 1) * 512],
                                         ps[:, :], AF.Copy)
            # V rows (group slice): V[jc] = sum_din xT[din, jc-slice]^T wkvv[din]
            for jc in range(NT):
                ps = psS.tile([128, G * DH], F32, tag="pss")
                for dc in range(8):
                    nc.tensor.matmul(
                        ps[:, :], xT_sb[:, dc, jc * 128:(jc + 1) * 128],
                        wkvv_sb[:, dc, :],
                        start=(dc == 0), stop=(dc == 7))
                nc.scalar.activation(V_sb[:, jc + 1, :], ps[:, :], AF.Copy)
            ppool_cm.__exit__(None, None, None)

            from contextlib import ExitStack
            stack = ExitStack()
            nb = 5 if batch_dots else 3
            wpool = stack.enter_context(tc.tile_pool(name="work", bufs=3))
            dpool = stack.enter_context(tc.tile_pool(name="dots", bufs=nb))
            apool = stack.enter_context(tc.tile_pool(name="attn", bufs=4))
            mpool = stack.enter_context(tc.tile_pool(name="maskp", bufs=3))
            spool = stack.enter_context(tc.tile_pool(name="small", bufs=3))
            drampool = stack.enter_context(tc.tile_pool(name="dramsc", bufs=3, space="DRAM"))

            # ---------- main loop over row tiles ----------
            import os as _os
            _order = list(range(NT))
            if _os.environ.get("KB_ORDER") == "rev":
                _order = list(range(1, NT)) + [0]
            elif _os.environ.get("KB_ORDER") == "big":
                _order = list(range(NT - 1, -1, -1))
            for t in _order:
                jl = jlen_of(t)
                tc0, tc1 = t * 128, (t + 1) * 128

                # Q''_k^T for the 4 group heads (bf16, scaled by pp*SCALE)
                qpp = wpool.tile([128, G, 8, 128], BF16, tag="qpp")
                for m in range(8):
                    for g in range(G):
                        nc.gpsimd.tensor_scalar_mul(
                            qpp[:, g, m, :], qT_sb[:, m, tc0:tc1],
                            ppv_sb[:, m * G + g: m * G + g + 1])

                aoT = wpool.tile([128, 2, 128], BF16, tag="aoT")

                def emit_dots(g):
                    dots = dpool.tile([128, jlen_of(NT - 1)], F32, tag="dots")
                    nj = (jl + 511) // 512
                    for jq in range(nj):
                        w = min(512, jl - jq * 512)
                        ps = psD.tile([128, 512], F32, tag="psd")
                        for m in range(8):
                            nc.tensor.matmul(
                                ps[:, :w], qpp[:, g, m, :],
                                KT_sb[:, m, jq * 512: jq * 512 + w],
                                start=(m == 0), stop=(m == 7))
                        nc.scalar.activation(dots[:, jq * 512: jq * 512 + w],
                                             ps[:, :w], AF.Copy)
                    # causal band add on last 128 cols
                    nc.vector.tensor_tensor(dots[:, jl - 128: jl],
                                            dots[:, jl - 128: jl],
                                            bandm_sb[:, :], ALU.add)
                    return dots

                pre = {g: emit_dots(g) for g in range(G)} if batch_dots else {}
                for g in range(G):
                    dots = pre[g] if batch_dots else emit_dots(g)

                    # ---- top-64 threshold ----
                    m8 = spool.tile([128, 64], F32, tag="m8")
                    mx8 = spool.tile([128, 8], F32, tag="mx8")
                    if t <= 2:
                        nc.vector.max(mx8[:, :], dots[:, :jl])
                        scr = mpool.tile([128, jlen_of(2)], F32, tag="scr")
                        src = dots
                        for r in range(8):
                            nc.vector.max(m8[:, r * 8:(r + 1) * 8], src[:, :jl])
                            nc.vector.match_replace(scr[:, :jl], m8[:, r * 8:(r + 1) * 8],
                                                    src[:, :jl], NEG)
                            src = scr
                    else:
                        L = 32 if t <= 6 else 64
                        S = (jl + L - 1) // L
                        cand = spool.tile([128, 8 * 33], F32, tag="cand")
                        for s in range(S):
                            w = min(L, jl - s * L)
                            nc.vector.max(cand[:, 8 * s: 8 * s + 8],
                                          dots[:, s * L: s * L + w])
                        W = 8 * S
                        nc.vector.max(mx8[:, :], cand[:, :W])
                        for r in range(8):
                            nc.vector.max(m8[:, r * 8:(r + 1) * 8], cand[:, :W])
                            nc.vector.match_replace(cand[:, :W], m8[:, r * 8:(r + 1) * 8],
                                                    cand[:, :W], NEG)
                    kth = m8[:, TOPK - 1: TOPK]
                    negmax = spool.tile([128, 1], F32, tag="negmax")
                    nc.vector.tensor_scalar_mul(negmax, mx8[:, 0:1], -1.0)

                    # ---- masked softmax ----
                    mask01 = mpool.tile([128, jlen_of(NT - 1)], BF16, tag="mask01")
                    nc.gpsimd.tensor_scalar(mask01[:, :jl], dots[:, :jl], kth, None,
                                            op0=ALU.is_ge)
                    attn = apool.tile([128, jlen_of(NT - 1)], BF16, tag="attn")
                    nc.scalar.activation(attn[:, :jl], dots[:, :jl], AF.Exp,
                                         bias=negmax[:, :])
                    # Z from the extracted top-64 values
                    e64 = spool.tile([128, 64], BF16, tag="e64")
                    zsum = spool.tile([128, 1], F32, tag="zsum")
                    nc.scalar.activation(e64[:, :], m8[:, :], AF.Exp,
                                         bias=negmax[:, :], accum_out=zsum[:, :])
                    rz = spool.tile([128, 1], F32, tag="rz")
                    nc.vector.reciprocal(rz, zsum)
                    # attn = (attn * rz) * mask01
                    nc.vector.scalar_tensor_tensor(attn[:, :jl], attn[:, :jl], rz,
                                                   mask01[:, :jl],
                                                   op0=ALU.mult, op1=ALU.mult)

                    # ---- attn^T (DMA transpose for seq chunks, PE for mem) ----
                    attnT = wpool.tile([128, t + 2, 128], BF16, tag="attnT")
                    pmem = psS.tile([16, 128], BF16, tag="pss")
                    nc.tensor.transpose(pmem[:, :], attn[:, 0:M], ident_sb[:, :])
                    nc.scalar.activation(attnT[0:M, 0, :], pmem[:, :], AF.Copy)
                    if use_dmat == "dram":
                        adram = drampool.tile([128, N], BF16, tag="adram")
                        nc.sync.dma_start(adram[:, : jl - M], attn[:, M:jl])
                        for c in range(t + 1):
                            nc.sync.dma_start_transpose(
                                attnT[:, c + 1, :], adram[:, c * 128:(c + 1) * 128])
                    elif use_dmat:
                        for c in range(t + 1):
                            nc.sync.dma_start_transpose(
                                attnT[:, c + 1, :], attn[:, M + c * 128: M + (c + 1) * 128])
                    else:
                        for c in range(t + 1):
                            pt = psS.tile([128, 128], BF16, tag="pss")
                            nc.tensor.transpose(pt[:, :], attn[:, M + c * 128: M + (c + 1) * 128],
                                                ident_sb[:, :])
                            nc.scalar.activation(attnT[:, c + 1, :], pt[:, :], AF.Copy)

                    # ---- out^T_g = V^T @ attn^T -> [64 d, 128 i] ----
                    po = psO.tile([64, 128], F32, tag="po")
                    nc.tensor.matmul(po[:, :], V_sb[0:M, 0, g * DH:(g + 1) * DH],
                                     attnT[0:M, 0, :], start=True, stop=False)
                    for c in range(t + 1):
                        nc.tensor.matmul(po[:, :], V_sb[:, c + 1, g * DH:(g + 1) * DH],
                                         attnT[:, c + 1, :],
                                         start=False, stop=(c == t))
                    nc.scalar.activation(aoT[(g % 2) * 64:(g % 2) * 64 + 64, g // 2, :],
                                         po[:, :], AF.Copy)

                # ---- partial y^T for this tile ----
                for dc in range(8):
                    ps = psS.tile([128, 128], F32, tag="pss")
                    for kc in range(2):
                        nc.tensor.matmul(ps[:, :], wout_sb[:, kc, dc * 128:(dc + 1) * 128],
                                         aoT[:, kc, :], start=(kc == 0), stop=(kc == 1))
                    yt = spool.tile([128, 128], BF16, tag="yt")
                    nc.scalar.activation(yt[:, :], ps[:, :], AF.Copy)
                    nc.sync.dma_start(ytp[t, dc], yt[:, :])
                # chunked all-reduce over the 4-core batch group
                if use_cc:
                    nc.gpsimd.collective_compute(
                        "AllReduce", ALU.add,
                        replica_groups=[[0, 1, 2, 3], [4, 5, 6, 7]],
                        ins=[ytp[t]],
                        outs=[ytr[t]])
                else:
                    nc.sync.dma_start(ytr[t], ytp[t])

                # ---- finale for this tile: transpose back, add bias, store
                ysb = wpool.tile([128, 8, 128], BF16, tag="ysb")
                for dc in range(8):
                    if use_dmat_fin:
                        nc.sync.dma_start_transpose(ysb[:, dc, :], ytr[t, dc])
                    else:
                        ysf = spool.tile([128, 128], BF16, tag="ysf")
                        nc.sync.dma_start(ysf[:, :], ytr[t, dc])
                        pt2 = psS.tile([128, 128], BF16, tag="pss")
                        nc.tensor.transpose(pt2[:, :], ysf[:, :], ident_sb[:, :])
                        nc.scalar.activation(ysb[:, dc, :], pt2[:, :], AF.Copy)
                yf = wpool.tile([128, DIM], F32, tag="yf")
                nc.gpsimd.tensor_tensor(
                    yf[:, :], ysb.rearrange("p c f -> p (c f)"),
                    bout_sb[:, :], ALU.add)
                nc.sync.dma_start(y[t], yf[:, :])
            stack.close()
    return nc


_cache = {}


def _get_compiled(use_cc=True, use_dmat=True, use_dmat_fin=True):
    key = (use_cc, use_dmat, use_dmat_fin)
    if key not in _cache:
        nc = bacc.Bacc("TRN2", target_bir_lowering=False, debug=False, num_devices=8)
        build(nc, use_cc=use_cc, use_dmat=use_dmat, use_dmat_fin=use_dmat_fin)
        nc.compile()
        _cache[key] = nc
    return _cache[key]


def make_in_maps(x, Wq, Wkv, pre_proj, mem_k, mem_v, Wout, bout):
    bf = ml_dtypes.bfloat16
    x, Wq, Wkv = np.asarray(x, np.float32), np.asarray(Wq, np.float32), np.asarray(Wkv, np.float32)
    pre_proj, mem_k, mem_v = (np.asarray(a, np.float32) for a in (pre_proj, mem_k, mem_v))
    Wout, bout = np.asarray(Wout, np.float32), np.asarray(bout, np.float32)

    pps = pre_proj * SCALE
    pprep = np.repeat(pps, DH, axis=0)                       # [1024, 16]
    memkT_full = mem_k.transpose(0, 2, 1).reshape(H * DH, M)  # [(h,d), m]
    band = np.where(np.arange(128)[None, :] <= np.arange(128)[:, None],
                    0.0, NEG).astype(bf)
    identity = np.eye(128, dtype=np.float32)

    in_maps = []
    for c in range(8):
        b = c // 4
        g0 = (c % 4) * G
        in_maps.append({
            "xT": np.ascontiguousarray(x[b].T).astype(bf).reshape(8, 128, N),
            "wq": Wq.astype(bf).reshape(8, 128, DIM),
            "wkvk": Wkv[:, :H * DH].astype(bf).reshape(8, 128, DIM),
            "wkvv": Wkv[:, H * DH + g0 * DH: H * DH + (g0 + G) * DH].astype(bf).reshape(8, 128, G * DH),
            "memkT": memkT_full.astype(bf).reshape(8, 128, M),
            "memv": mem_v[g0:g0 + G].transpose(1, 0, 2).reshape(M, G * DH).astype(bf),
            "ppv": np.ascontiguousarray(pprep[:, g0:g0 + G]).reshape(8, 128, G).astype(np.float32),
            "wout": Wout[g0 * DH:(g0 + G) * DH].astype(bf).reshape(2, 128, DIM),
            "bout": np.broadcast_to(bout.reshape(1, DIM), (128, DIM)).astype(bf),
            "bandm": band,
            "ident": identity.astype(bf),
        })
    return in_maps


import os
def run_traced(inputs, trace=False, **trace_kwargs):
    nc = _get_compiled(use_cc=os.environ.get("KB_NO_CC") != "1",
                       use_dmat=os.environ.get("KB_DMAT") == "1",
                       use_dmat_fin=os.environ.get("KB_DMAT_FIN", "1") == "1")
    in_maps = make_in_maps(**inputs)
    res = run_bass_kernel_spmd(nc, in_maps, core_ids=list(range(8)),
                               trace=trace, **trace_kwargs)
    out = np.stack([res.results[0]["y"].reshape(N, DIM),
                    res.results[4]["y"].reshape(N, DIM)]).astype(np.float32)
    return out, res


def kernel(x, Wq, Wkv, pre_proj, mem_k, mem_v, Wout, bout):
    out, _ = run_traced(dict(x=x, Wq=Wq, Wkv=Wkv, pre_proj=pre_proj, mem_k=mem_k,
                             mem_v=mem_v, Wout=Wout, bout=bout))
    return out


if __name__ == "__main__":
    import sys, time
    sys.path.insert(0, "/root/problem")
    import reference as ref
    inputs = {k: np.asarray(v) for k, v in ref.setup_inputs().items()}
    expected = np.asarray(ref.reference(**inputs))
    actual = kernel(**inputs)
    err = np.linalg.norm(actual - expected) / np.linalg.norm(expected)
    print(f"rel_err={err:.3e} maxabs={np.max(np.abs(actual - expected)):.3e}", flush=True)
    times = []
    for _ in range(5):
        t0 = time.time()
        kernel(**inputs)
        times.append(time.time() - t0)
    print("wall times ms:", [f"{t*1e3:.1f}" for t in times], "min:", f"{min(times)*1e3:.1f}")
